# revision 3
# baseline (speedup 1.0000x reference)
"""GATv2 (2-layer) Trainium2 Bass kernel, 8-core SPMD — v3.

Feature-partitioned edge pipeline via TRANSPOSE-mode dma_gather (HW-verified
to sign-extend int16 idx offsets, see tgather_test.py):
  - gather puts edge slot s in the FREE dim, features in partitions
  - attention logit u = sign-matmul over |z| rows (+ raw q row), with a
    column-replicated lhsT so the PSUM result is already broadcast to all
    128 partitions; exp straight out of PSUM
  - weighted aggregation = DVE multiply + windowed reduce over each node's
    b contiguous slots — no spread DMAs, no per-chunk agg matmuls, no
    one-hot patterns
Payload: 3 input arrays (XT, IDX, WB ~ 16MB total), f16 OUT.
"""

import os

import numpy as np

import concourse.bacc as bacc
import concourse.bass as bass
import concourse.mybir as mybir
from concourse.bass_utils import run_bass_kernel_spmd
from concourse.library_config import mlp
from concourse.tile import TileContext

F16 = mybir.dt.float16
F32 = mybir.dt.float32
I16 = mybir.dt.int16
AF = mybir.ActivationFunctionType
AX = mybir.AxisListType
ALU = mybir.AluOpType

NCORE = 8
BUCKETS = (16, 32, 64)
POISON = -30000.0


# ---------------------------------------------------------------- structure
def build_plan(src, dst, n_nodes, ncore):
    npc = n_nodes // ncore
    deg = np.bincount(dst, minlength=n_nodes)
    assert deg.min() >= 1 and deg.max() <= BUCKETS[-1], (deg.min(), deg.max())
    bucket = np.full(n_nodes, BUCKETS[-1], np.int64)
    for b in reversed(BUCKETS):
        bucket[deg <= b] = b
    core_of = np.arange(n_nodes) // npc

    ncap_b = {}
    for b in BUCKETS:
        cnt = int(max(((bucket == b) & (core_of == c)).sum()
                      for c in range(ncore)))
        cap = ((cnt + 127) // 128) * 128
        if b == BUCKETS[-1] and cap == cnt:
            cap += 128  # guarantee a dummy (poison) slot at the very end
        ncap_b[b] = cap
    ncap = sum(ncap_b.values())
    ng = ncore * ncap
    gbase = ng // 2
    assert ng <= 65534, ng
    padidx = gbase - 1  # row ng-1: per-core poison row (dummy, forced below)

    tiles = []
    pos = 0
    for b in BUCKETS:
        for t in range(ncap_b[b] // 128):
            tiles.append((b, pos + t * 128))
        pos += ncap_b[b]
    totc = sum(b for b, _ in tiles)

    order = np.full((ncore, ncap), -1, np.int64)
    grow = np.full(n_nodes, -1, np.int64)   # global table row of node
    for c in range(ncore):
        pos = 0
        for b in BUCKETS:
            nodes = np.where((bucket == b) & (core_of == c))[0]
            order[c, pos:pos + len(nodes)] = nodes
            grow[nodes] = c * ncap + pos + np.arange(len(nodes))
            pos += ncap_b[b]
    assert (order[:, ncap - 1] == -1).all(), "poison slot must be dummy"

    es = np.argsort(dst, kind="stable")
    ssrc = src[es]
    starts = np.zeros(n_nodes + 1, np.int64)
    np.cumsum(deg, out=starts[1:])

    idx16 = np.zeros((ncore, totc * 128), np.int16)

    run_start = np.zeros(totc * 128, np.int64)
    run_b = np.zeros(totc * 128, np.int64)
    for c in range(ncore):
        slot = 0
        for (b, p0) in tiles:
            for j in range(128):
                node = order[c, p0 + j]
                run_start[slot:slot + b] = slot
                run_b[slot:slot + b] = b
                if node < 0:
                    idx16[c, slot:slot + b] = 0  # dummy: junk row, w finite
                    slot += b
                    continue
                d = deg[node]
                rows = (grow[ssrc[starts[node]:starts[node] + d]] - gbase)
                rows = np.sort(rows)  # negatives first, non-negatives last
                idx16[c, slot:slot + d] = rows.astype(np.int16)
                idx16[c, slot + d:slot + b] = padidx  # poison row: w = 0
                slot += b
        assert slot == totc * 128
        # HW drops TRAILING negative idxs within each 512-idx transpose
        # gather, so every 512-block must END >= 0.  Slot order within one
        # node's run is free -> swap a non-negative (or pad) idx onto the
        # boundary.  A node spans < 512 slots, so each run holds <= 1
        # boundary and swaps never collide.
        for blk in range(511, totc * 128, 512):
            if idx16[c, blk] >= 0:
                continue
            s0 = run_start[blk]
            hi = s0 + run_b[blk] - 1
            assert idx16[c, hi] >= 0, (c, blk, "all-negative full node")
            idx16[c, blk], idx16[c, hi] = idx16[c, hi], idx16[c, blk]

    idxw = np.zeros((ncore, 16, totc * 8), np.int16)
    for c in range(ncore):
        idxw[c] = idx16[c].reshape(totc * 8, 16).T

    return dict(deg=deg, bucket=bucket, ncap_b=ncap_b, ncap=ncap, ng=ng,
                gbase=gbase, tiles=tiles, totc=totc, nt=len(tiles),
                order=order, grow=grow, idxw=idxw)


# ---------------------------------------------------------------- weights
def prep_weights(W1_l, W1_r, b1_l, b1_r, a1, bias1, W2_l, W2_r, b2_l, b2_r,
                 a2, bias2):
    """Sign-permute features, fold a into tables; pack all consts into WB."""
    p1 = np.argsort(a1 < 0, kind="stable")     # a1>=0 first
    n1p = int((a1 >= 0).sum())
    a1p = a1[p1]
    W1_lp, W1_rp = W1_l[:, p1], W1_r[:, p1]
    b1_lp, b1_rp = b1_l[p1], b1_r[p1]
    bias1p = bias1[p1]
    p2 = np.argsort(a2 < 0, kind="stable")
    n2p = int((a2 >= 0).sum())
    a2p = a2[p2]
    W2_lp, W2_rp = W2_l[p1][:, p2], W2_r[p1][:, p2]
    b2_lp, b2_rp = b2_l[p2], b2_r[p2]
    bias2p = bias2[p2]

    w1pack = np.concatenate([
        W1_lp * a1p[None, :], 1.5 * (W1_lp @ a1p)[:, None],
        W1_rp * a1p[None, :], 1.5 * (W1_rp @ a1p)[:, None]], axis=1)  # [128,130]
    b1pack = np.concatenate([
        b1_lp * a1p, [1.5 * (b1_lp @ a1p)],
        b1_rp * a1p, [1.5 * (b1_rp @ a1p)]])                          # [130]
    w2pack = np.concatenate([
        W2_lp * a2p[None, :], 1.5 * (W2_lp @ a2p)[:, None],
        W2_rp * a2p[None, :], 1.5 * (W2_rp @ a2p)[:, None]], axis=1)  # [64,34]
    b2pack = np.concatenate([
        b2_lp * a2p, [1.5 * (b2_lp @ a2p)],
        b2_rp * a2p, [1.5 * (b2_rp @ a2p)]])                          # [34]

    wb = np.zeros((128, 182), np.float16)
    wb[:, 0:65] = w1pack[:, 0:65].astype(np.float16)      # w1l (vals + ql)
    wb[:, 65:130] = w1pack[:, 65:130].astype(np.float16)  # w1r
    wb[0:64, 130:164] = w2pack.astype(np.float16)
    wb[0:65, 164] = b1pack[0:65].astype(np.float16)       # b1l col
    wb[0:65, 165] = b1pack[65:130].astype(np.float16)     # b1r col
    wb[0:17, 166] = b2pack[0:17].astype(np.float16)       # b2l col
    wb[0:17, 167] = b2pack[17:34].astype(np.float16)      # b2r col
    wb[0:64, 168] = np.where(np.arange(64) < n1p, 1.0, -1.0)  # Ls1
    wb[64, 169] = 1.0                                      # Lq1 (e64)
    wb[0:16, 170] = np.where(np.arange(16) < n2p, 1.0, -1.0)  # Ls2
    wb[16, 171] = 1.0                                      # Lq2 (e16)
    wb[0:64, 172:174] = (1.0 / a1p).astype(np.float32)[:, None].view(np.float16)
    wb[0:64, 174:176] = bias1p.astype(np.float32)[:, None].view(np.float16)
    wb[0:16, 176:178] = (1.0 / a2p).astype(np.float32)[:, None].view(np.float16)
    wb[0:16, 178:180] = bias2p.astype(np.float32)[:, None].view(np.float16)
    wb[64, 180] = POISON                                   # poison1 col (q@64)
    wb[16, 181] = POISON                                   # poison2 col (q@16)
    return dict(p1=p1, p2=p2, n1p=n1p, n2p=n2p, wb=wb.view(np.int16))


# ---------------------------------------------------------------- device
def build_program(plan, wp, ncore):
    ncap, nt, totc, gbase = plan["ncap"], plan["nt"], plan["totc"], plan["gbase"]
    tiles = plan["tiles"]
    ng = plan["ng"]

    nc = bacc.Bacc("TRN2", num_swdge_queues=1)
    XT = nc.declare_dram_parameter("XT", [128, ncap], F16, isOutput=False)
    IDX = nc.declare_dram_parameter("IDX", [16, totc * 8], I16, isOutput=False)
    WB = nc.declare_dram_parameter("WB", [128, 182], I16, isOutput=False)
    OUT = nc.declare_dram_parameter("OUT", [ncap, 16], F16, isOutput=True)

    T1s = nc.dram_tensor("T1s", [ncap, 65], F16)
    shared = "Shared" if ncore > 4 else "Local"
    T1fc = nc.dram_tensor("T1fc", [ng, 65], F16, addr_space=shared)
    T1f = nc.dram_tensor("T1f", [ng, 128], F16)
    T2s = nc.dram_tensor("T2s", [ncap, 17], F16)
    T2fc = nc.dram_tensor("T2fc", [ng, 17], F16, addr_space=shared)
    T2f = nc.dram_tensor("T2f", [ng, 128], F16)

    with TileContext(nc) as tc:
        nc.gpsimd.load_library(mlp)
        with tc.tile_pool(name="const", bufs=1) as cpool, \
             tc.tile_pool(name="work", bufs=2) as pool, \
             tc.tile_pool(name="zpool", bufs=6) as zpool, \
             tc.tile_pool(name="pa", bufs=2, space="PSUM") as papool, \
             tc.tile_pool(name="pe", bufs=1, space="PSUM") as pepool:

            wb = cpool.tile([128, 182], I16)
            nc.sync.dma_start(wb[:], WB[:])
            idxs = cpool.tile([128, totc * 8], I16)
            for g in range(8):
                nc.scalar.dma_start(idxs[16 * g:16 * (g + 1), :], IDX[:, :])
            w1l = wb[:, 0:65].bitcast(F16)
            w1r = wb[:, 65:130].bitcast(F16)
            w2p = wb[0:64, 130:164].bitcast(F16)
            b1l = wb[0:65, 164:165].bitcast(F16)
            b1r = wb[0:65, 165:166].bitcast(F16)
            b2l = wb[0:17, 166:167].bitcast(F16)
            b2r = wb[0:17, 167:168].bitcast(F16)
            ls1 = wb[0:64, 168:169].bitcast(F16)
            lq1 = wb[0:65, 169:170].bitcast(F16)
            ls2 = wb[0:16, 170:171].bitcast(F16)
            lq2 = wb[0:17, 171:172].bitcast(F16)
            inv1 = wb[0:64, 172:174].bitcast(F32)
            bs1 = wb[0:64, 174:176].bitcast(F32)
            inv2 = wb[0:16, 176:178].bitcast(F32)
            bs2 = wb[0:16, 178:180].bitcast(F32)
            poi1 = wb[0:65, 180:181].bitcast(F16)
            poi2 = wb[0:17, 181:182].bitcast(F16)

            # column-replicated sign/qpick lhsT mats (PSUM result = broadcast)
            lrep1 = cpool.tile([64, 128], F16)
            nc.vector.tensor_scalar(lrep1[:], ls1.broadcast_to([64, 128]),
                                    1.0, 0.0, ALU.mult, ALU.add)
            lqrep1 = cpool.tile([65, 128], F16)
            nc.vector.tensor_scalar(lqrep1[:], lq1.broadcast_to([65, 128]),
                                    1.0, 0.0, ALU.mult, ALU.add)
            lrep2 = cpool.tile([16, 128], F16)
            nc.vector.tensor_scalar(lrep2[:], ls2.broadcast_to([16, 128]),
                                    1.0, 0.0, ALU.mult, ALU.add)
            lqrep2 = cpool.tile([17, 128], F16)
            nc.vector.tensor_scalar(lqrep2[:], lq2.broadcast_to([17, 128]),
                                    1.0, 0.0, ALU.mult, ALU.add)

            xrtT = cpool.tile([65, nt * 128], F16)    # layer-1 r-side, [f, node]
            h2rtT = cpool.tile([17, nt * 128], F16)   # layer-2 r-side
            stg_a = cpool.tile([65, 128], F16)
            stg_b = cpool.tile([65, 128], F16)
            stg2 = [stg_a, stg_b]
            r512 = nc.gpsimd.to_reg(512)

            t1v = T1s[:, :]
            t2v = T2s[:, :]
            outv = OUT[:, :]

            # ---------------- phase A: layer-1 tables (transposed) ----------
            for t in range(nt):
                xtc = pool.tile([128, 128], F16, tag="xtc")
                nc.sync.dma_start(xtc[:], XT[:, t * 128:(t + 1) * 128])
                psl = pepool.tile([65, 128], F32, tag="pA")
                nc.tensor.matmul(psl[:], w1l, xtc[:], start=True, stop=True)
                psr = pepool.tile([65, 128], F32, tag="pA2")
                nc.tensor.matmul(psr[:], w1r, xtc[:], start=True, stop=True)
                stgT = stg2[t % 2]
                nc.vector.tensor_add(stgT[:], psl[:],
                                     b1l.broadcast_to([65, 128]))
                nc.vector.tensor_add(xrtT[:, t * 128:(t + 1) * 128], psr[:],
                                     b1r.broadcast_to([65, 128]))
                dstT = bass.AP(t1v.tensor, int(t1v.offset) + t * 128 * 65,
                               [[1, 65], [65, 128]])
                nc.sync.dma_start(dstT, stgT[:])
            # poison row (padidx target): zeros except q col = -30000
            nc.sync.dma_start(
                bass.AP(t1v.tensor, int(t1v.offset) + (ncap - 1) * 65,
                        [[1, 65]]),
                poi1)
            if not os.environ.get("GAT_SKIP_CC"):
                nc.gpsimd.collective_compute(
                    "AllGather", ALU.bypass,
                    replica_groups=[list(range(ncore))],
                    ins=[T1s[:]], outs=[T1fc[:]])
            else:
                nc.sync.dma_start(T1fc[0:ncap, :], T1s[:, :])
            nc.sync.dma_start(T1f[:, 0:65], T1fc[:, :])

            # ---------------- per-layer edge phase ----------------
            def layer(F, Tf, xrT, lrep, lqrep, emit):
                tfv = Tf[:, :]
                scol = 0   # idx column offset (slots/16)
                for ti, (b, p0) in enumerate(tiles):
                    nm = pool.tile([64, 128], F32, tag="nm")
                    dn = pool.tile([64, 128], F32, tag="dn")
                    subs = [(0, 128)] if b <= 32 else [(0, 64), (64, 64)]
                    assert all(nn_ * b % 512 == 0 and nn_ * b <= 4096
                               for _, nn_ in subs), (b, subs)
                    for (prow, nn) in subs:
                        S = nn * b
                        zT = zpool.tile([128, 4096], F16, tag="zt")
                        if os.environ.get("GAT_SKIP_GATHER"):
                            nc.vector.memset(zT[:, 0:S], 0.125)
                        else:
                            for g0 in range(0, S, 512):
                                nc.gpsimd.dma_gather(
                                    zT[:, g0:g0 + 512].rearrange(
                                        "p (o i) -> p o i", o=1),
                                    Tf[gbase:, :],
                                    idxs[:, scol + g0 // 16:
                                         scol + (g0 + 512) // 16],
                                    512, r512, 128, transpose=True)
                        zS = pool.tile([65, 4096], F16, tag="zs")
                        xv = xrT[0:F + 1,
                                 ti * 128 + prow:ti * 128 + prow + nn]
                        nc.vector.tensor_add(
                            zS[0:F + 1, 0:S].rearrange("p (m e) -> p m e",
                                                       e=b),
                            zT[0:F + 1, 0:S].rearrange("p (m e) -> p m e",
                                                       e=b),
                            xv.unsqueeze(2).broadcast_to([F + 1, nn, b]))
                        azS = pool.tile([64, 4096], F16, tag="az")
                        nc.scalar.activation(azS[0:F, 0:S], zS[0:F, 0:S],
                                             AF.Abs)
                        exS = pool.tile([128, 4096], F16, tag="ex")
                        for g0 in range(0, S, 1024):
                            gw = min(1024, S - g0)
                            psU = papool.tile([128, 1024], F32, tag="psU")
                            for h0 in range(0, gw, 512):
                                nc.tensor.matmul(psU[:, h0:h0 + 512],
                                                 lrep[0:F, :],
                                                 azS[0:F, g0 + h0:
                                                     g0 + h0 + 512],
                                                 start=True, stop=False)
                                nc.tensor.matmul(psU[:, h0:h0 + 512],
                                                 lqrep[0:F + 1, :],
                                                 zS[0:F + 1, g0 + h0:
                                                    g0 + h0 + 512],
                                                 start=False, stop=True)
                            nc.scalar.activation(exS[:, g0:g0 + gw],
                                                 psU[:, 0:gw],
                                                 AF.Exp, scale=0.4)
                        nc.vector.reduce_sum(
                            dn[0:F, prow:prow + nn],
                            exS[0:F, 0:S].rearrange("p (m e) -> p m e", e=b),
                            axis=AX.X)
                        nmul = pool.tile([64, 4096], F32, tag="nmul")
                        nc.vector.tensor_mul(nmul[0:F, 0:S], zT[0:F, 0:S],
                                             exS[0:F, 0:S])
                        nc.vector.reduce_sum(
                            nm[0:F, prow:prow + nn],
                            nmul[0:F, 0:S].rearrange("p (m e) -> p m e", e=b),
                            axis=AX.X)
                        scol += S // 16
                    emit(ti, nm, dn)

            def emit1(ti, nm, dn):
                rden = pool.tile([64, 128], F32, tag="rden")
                nc.vector.reciprocal(rden[:], dn[0:64, :])
                h1 = pool.tile([64, 128], F32, tag="h1")
                nc.vector.tensor_mul(h1[:], nm[0:64, :], rden[:])
                nc.vector.tensor_mul(h1[:], h1[:],
                                     inv1.broadcast_to([64, 128]))
                nc.vector.tensor_add(h1[:], h1[:],
                                     bs1.broadcast_to([64, 128]))
                h = pool.tile([64, 128], F16, tag="h")
                nc.scalar.activation(h[:], h1[:], AF.Relu)
                ps2l = pepool.tile([17, 128], F32, tag="tmp")
                nc.tensor.matmul(ps2l[:], w2p[:, 0:17], h[:],
                                 start=True, stop=True)
                ps2r = pepool.tile([17, 128], F32, tag="tmp2")
                nc.tensor.matmul(ps2r[:], w2p[:, 17:34], h[:],
                                 start=True, stop=True)
                stgT = pool.tile([17, 128], F16, tag="s2g")
                nc.vector.tensor_add(stgT[:], ps2l[:],
                                     b2l.broadcast_to([17, 128]))
                nc.vector.tensor_add(h2rtT[:, ti * 128:(ti + 1) * 128],
                                     ps2r[:],
                                     b2r.broadcast_to([17, 128]))
                dstT = bass.AP(t2v.tensor, int(t2v.offset) + ti * 128 * 17,
                               [[1, 17], [17, 128]])
                nc.sync.dma_start(dstT, stgT[:])

            def emit2(ti, nm, dn):
                rden = pool.tile([64, 128], F32, tag="rden")
                nc.vector.reciprocal(rden[0:16, :], dn[0:16, :])
                o1 = pool.tile([16, 128], F32, tag="o1")
                nc.vector.tensor_mul(o1[:], nm[0:16, :], rden[0:16, :])
                nc.vector.tensor_mul(o1[:], o1[:],
                                     inv2.broadcast_to([16, 128]))
                o2 = pool.tile([16, 128], F16, tag="o2")
                nc.vector.tensor_add(o2[:], o1[:],
                                     bs2.broadcast_to([16, 128]))
                dstT = bass.AP(outv.tensor, int(outv.offset) + ti * 128 * 16,
                               [[1, 16], [16, 128]])
                nc.sync.dma_start(dstT, o2[:])

            layer(64, T1f, xrtT, lrep1, lqrep1, emit1)
            nc.sync.dma_start(
                bass.AP(t2v.tensor, int(t2v.offset) + (ncap - 1) * 17,
                        [[1, 17]]),
                poi2)
            if not os.environ.get("GAT_SKIP_CC"):
                nc.gpsimd.collective_compute(
                    "AllGather", ALU.bypass,
                    replica_groups=[list(range(ncore))],
                    ins=[T2s[:]], outs=[T2fc[:]])
            else:
                nc.sync.dma_start(T2fc[0:ncap, :], T2s[:, :])
            nc.sync.dma_start(T2f[:, 0:17], T2fc[:, :])
            layer(16, T2f, h2rtT, lrep2, lqrep2, emit2)

    nc.compile()
    return nc


# ---------------------------------------------------------------- host entry
def kernel(x, edge_index, W1_l, W1_r, b1_l, b1_r, a1, bias1,
           W2_l, W2_r, b2_l, b2_r, a2, bias2, _run=None, _ncore=NCORE):
    x = np.asarray(x, np.float32)
    ei = np.asarray(edge_index)
    n = x.shape[0]
    loop = np.arange(n, dtype=ei.dtype)
    src = np.concatenate([np.asarray(ei[0]), loop]).astype(np.int64)
    dst = np.concatenate([np.asarray(ei[1]), loop]).astype(np.int64)

    plan = build_plan(src, dst, n, _ncore)
    wp = prep_weights(*[np.asarray(a, np.float32) for a in
                        (W1_l, W1_r, b1_l, b1_r, a1, bias1,
                         W2_l, W2_r, b2_l, b2_r, a2, bias2)])
    nc = build_program(plan, wp, _ncore)

    in_maps = []
    for c in range(_ncore):
        xt = np.zeros((128, plan["ncap"]), np.float16)
        ordc = plan["order"][c]
        valid = ordc >= 0
        xt[:, np.where(valid)[0]] = x[ordc[valid]].T.astype(np.float16)
        in_maps.append({"XT": xt, "IDX": plan["idxw"][c], "WB": wp["wb"]})

    if _run is None:
        import time as _time
        res = run_bass_kernel_spmd(nc, in_maps, list(range(_ncore)))
        outs = [r["OUT"] for r in res.results]
        if os.environ.get("GAT_TRACE"):
            ts = []
            for _ in range(3):
                t0 = _time.time()
                run_bass_kernel_spmd(nc, in_maps, list(range(_ncore)))
                ts.append(_time.time() - t0)
            # min wall of a cached re-dispatch (includes host<->device I/O)
            print(f"HW exec time: {int(min(ts) * 1e9)} ns (e2e dispatch wall, "
                  f"runs: {[f'{t:.3f}s' for t in ts]})")
    else:
        outs = _run(nc, in_maps)   # test hook: returns list of OUT per core

    out = np.zeros((n, 16), np.float32)
    for c in range(_ncore):
        ordc = plan["order"][c]
        valid = ordc >= 0
        out[ordc[valid]] = np.asarray(outs[c], np.float32)[np.where(valid)[0]]
    inv_p2 = np.argsort(wp["p2"])
    return out[:, inv_p2].astype(np.float32)
